# revision 10
# baseline (speedup 1.0000x reference)
"""CPCNet forward on 8 Trainium2 NeuronCores (Bass/Tile).

Data-parallel over batch: each of the 8 cores processes 16 of the 128
batch elements end-to-end (embed GEMM -> GRU over 16 context windows ->
bilinear scoring), parameters replicated. No collectives needed.

Per-core layout (all "transposed" space, embed dim on partitions):
  rows = flattened [C*T]-windows: Xc 256 (s*16+b), Xp 256 (s*16+b),
  Xb 2560 (nb*256 + s*16 + b).  ET[sbuf] = [101, 3072] embeddings^T
  (row 100 = ones for bias folding via augmented matmuls).

Embed GEMM (the memory-bound bulk, ~103 MB/core): X streams in natural
layout [128 rows, k] (contiguous DMA rows) and is cast f32->bf16 inside
the SWDGE load DMAs; PE transposes 128x128 bf16 blocks into PSUM; DVE
evacuates transposed pairs to SBUF; PE accumulates W.T @ X^T into
E^T[100, 512] per 512-row block in bf16.

v2 scheduling (vs 377us baseline):
- Pool/GpSimd queue carries ONLY the SWDGE load triggers (GRU work on
  it was stalling descriptor generation -> DMA concurrency dips).
- psE (embed accumulator) has 2 PSUM bufs so consecutive 512-row blocks
  don't serialize on the evacuation; psT has 4.
- GRU gates accumulate in PSUM: gi (all 16 steps) computed once into
  psGI with biases folded via a ones-row in ET/h; per step the gh r/z
  matmuls accumulate straight onto gi (start=False) so ACT applies
  sigmoid directly from PSUM. Elementwise on DVE via fused
  scalar_tensor_tensor; sigmoid/tanh on the otherwise-idle ACT.
- One GRU step per k-slab gap across blocks 1-4 -> chain latency hides
  under the DMA stream, no queue head-of-line blocking.
- Bilinear scores computed per block as embeddings land (A = Wbil.T @ h
  precomputed after step 15); block k's scores issue right after its
  PSUM evacuation, so the post-stream tail is only the last slab's
  pipeline (~7us) instead of the whole GRU+bilinear (~33us).
- Block 0 loads its smallest k-slab (208 cols) first for a fast
  pipeline start (W chunks for it arrive via a separate small DMA);
  block 5 loads it last for a small tail.
"""

import numpy as np

import concourse.bacc as bacc
import concourse.mybir as mybir
import concourse.tile as tile
from concourse.bass_utils import run_bass_kernel_spmd

N_CORES = 8
BC = 16          # batch per core
NE = 16          # context windows (gru seq len)
NB = 10          # negative samples
CT = 8400        # flattened window (21*400)
E = 100          # embed dim == gru hidden
ROWS = BC * NE * (2 + NB)   # 3072 rows per core
NBLK = ROWS // 512          # 6 blocks of 512 rows
SLAB_SMALL = (8192, 208)
SLABS_BIG = [(0, 2048), (2048, 2048), (4096, 2048), (6144, 2048)]
NCHUNK = 66                 # ceil(8400/128); last chunk is 80 wide
NOUT = BC * NE * (NB + 1)   # 2816 score columns per core

F32 = mybir.dt.float32
F32R = mybir.dt.float32r
BF16 = mybir.dt.bfloat16


def _block_src(Xc, Xp, Xb, blk, st, k0, kw):
    """DRAM source AP for 128-row subtile `st` of 512-row block `blk`,
    k-range [k0, k0+kw). Row order within subtile: (s, b), s-major."""
    if blk == 0:
        base = Xc if st < 2 else Xp
        sh = st % 2
        return base[:, sh * 8:(sh + 1) * 8, k0:k0 + kw].transpose([1, 0, 2])
    nb = 2 * (blk - 1) + st // 2
    sh = st % 2
    return Xb[:, sh * 8:(sh + 1) * 8, nb, k0:k0 + kw].transpose([1, 0, 2])


def _emit(nc, tc, ctx):
    AOT = mybir.AluOpType
    ACTF = mybir.ActivationFunctionType

    Xc = nc.dram_tensor("Xc", [BC, NE, CT], F32, kind="ExternalInput").ap()
    Xp = nc.dram_tensor("Xp", [BC, NE, CT], F32, kind="ExternalInput").ap()
    Xb = nc.dram_tensor("Xb", [BC, NE, NB, CT], F32, kind="ExternalInput").ap()
    Wemb = nc.dram_tensor("Wemb", [128, NCHUNK * E], BF16,
                          kind="ExternalInput").ap()
    bemb = nc.dram_tensor("bemb", [E, 1], F32, kind="ExternalInput").ap()
    WihT = nc.dram_tensor("WihT", [E + 1, 300], F32, kind="ExternalInput").ap()
    WhhT = nc.dram_tensor("WhhT", [E + 1, 300], F32, kind="ExternalInput").ap()
    Wbil = nc.dram_tensor("Wbil", [E, NE * E], F32, kind="ExternalInput").ap()
    ident = nc.dram_tensor("ident", [128, 128], BF16, kind="ExternalInput").ap()
    ones = nc.dram_tensor("ones", [E, 1], F32R, kind="ExternalInput").ap()
    onesrow = nc.dram_tensor("onesrow", [1, ROWS], F32,
                             kind="ExternalInput").ap()
    out_d = nc.dram_tensor("out", [1, NOUT], F32, kind="ExternalOutput").ap()

    P = ctx.enter_context  # pools

    const = P(tc.tile_pool(name="const", bufs=1))
    xnat = P(tc.tile_pool(name="xnat", bufs=5))
    xtp = P(tc.tile_pool(name="xt", bufs=4))
    psT = P(tc.tile_pool(name="psT", bufs=3, space="PSUM"))
    psE = P(tc.tile_pool(name="psE", bufs=1, space="PSUM"))
    psG = P(tc.tile_pool(name="psG", bufs=1, space="PSUM"))
    psR = P(tc.tile_pool(name="psR", bufs=1, space="PSUM"))
    small = P(tc.tile_pool(name="small", bufs=2))

    # ---- persistent SBUF ----
    # identity first: the very first transposes only need id_sb + one X slab
    id_sb = const.tile([128, 128], BF16)
    nc.sync.dma_start(id_sb[:], ident[:])
    # W_embed arrives pre-chunked [128, 66*100] and pre-cast to bf16 from
    # the host. The chunks for block 0's first (small) k-slab come in a
    # separate tiny DMA so the first embed matmuls never wait on the
    # full 1.7 MB load.
    W_sb = const.tile([128, NCHUNK * E], BF16)
    nc.sync.dma_start(W_sb[:, 64 * E:NCHUNK * E], Wemb[:, 64 * E:NCHUNK * E])
    nc.sync.dma_start(W_sb[:, 0:64 * E], Wemb[:, 0:64 * E])
    bemb_sb = const.tile([E, 1], F32)
    nc.scalar.dma_start(bemb_sb[:], bemb[:])
    WihT_sb = const.tile([E + 1, 300], F32)
    nc.scalar.dma_start(WihT_sb[:], WihT[:])
    WhhT_sb = const.tile([E + 1, 300], F32)
    nc.scalar.dma_start(WhhT_sb[:], WhhT[:])
    Wbil_sb = const.tile([E, NE * E], F32)
    nc.scalar.dma_start(Wbil_sb[:], Wbil[:])
    ones_sb = const.tile([E, 1], F32R)
    nc.scalar.dma_start(ones_sb[:], ones[:])

    ET = const.tile([E + 1, ROWS], F32)        # embeddings^T + ones row
    h = const.tile([E + 1, BC], F32)           # GRU hidden h^T + ones row
    A_sb = const.tile([E, NE * BC], F32)       # bilinear A[e, s*16+b]
    rz = const.tile([E, 2 * BC], F32)          # sigmoid(r|z) per step
    out_sb = const.tile([1, NOUT], F32)

    # engine memsets can't start at partition 100 (32-aligned starts only);
    # fill the ones rows by DMA instead
    nc.scalar.dma_start(ET[E:E + 1, :], onesrow[:, :])
    nc.vector.memset(h[0:E, :], 0.0)
    nc.scalar.dma_start(h[E:E + 1, :], onesrow[:, 0:BC])

    # Gate preactivations live in PSUM, ONE accumulated bank per gate:
    # PSUM accumulation (start=False) only sees addresses written since the
    # bank's last start=True matmul, so each of gr/gz gets exactly one
    # start=True (gru_init) and is only ever accumulated onto afterwards.
    # gn additionally hosts the per-step gh_n scratch (cols 256:272); it is
    # never PE-accumulated, only memory-read, so its start=True resets are
    # harmless.
    gr = psG.tile([E, 512], F32, tag="gr", name="gr")
    gz = psG.tile([E, 512], F32, tag="gz", name="gz")
    gn = psG.tile([E, 512], F32, tag="gn", name="gn")

    def gru_init():
        # gi preacts for all 16 steps in 3 gate matmuls; biases folded via
        # the ones row of ET (WihT row 100 = b_ih + b_hh for r,z; b_ih for n).
        for g, dst in enumerate((gr, gz, gn)):
            nc.tensor.matmul(dst[:, 0:NE * BC],
                             WihT_sb[:, g * E:(g + 1) * E], ET[:, 0:NE * BC],
                             start=True, stop=True, skip_group_check=True)

    def gru_step(s):
        c0 = s * BC
        # gh_r / gh_z accumulate straight onto gi in PSUM (b_hh folded at
        # init); gh_n lands in the scratch slice with b_hn folded via the
        # ones row of h (WhhT row 100 = [0, 0, b_hn]).
        for g, dst in ((0, gr), (1, gz)):
            nc.tensor.matmul(dst[:, c0:c0 + BC],
                             WhhT_sb[:, g * E:(g + 1) * E], h[:],
                             start=False, stop=True, skip_group_check=True)
        nc.tensor.matmul(gn[:, 256:256 + BC],
                         WhhT_sb[:, 2 * E:3 * E], h[:],
                         start=True, stop=True, skip_group_check=True)
        nc.scalar.activation(rz[:, 0:BC], gr[:, c0:c0 + BC], ACTF.Sigmoid)
        nc.scalar.activation(rz[:, BC:2 * BC], gz[:, c0:c0 + BC],
                             ACTF.Sigmoid)
        t2 = small.tile([E, BC], F32, tag="t2", name="t2")
        nc.vector.scalar_tensor_tensor(t2[:], gn[:, 256:256 + BC], 1.0,
                                       rz[:, 0:BC], AOT.mult, AOT.mult)
        t3 = small.tile([E, BC], F32, tag="t3", name="t3")
        nc.vector.tensor_add(t3[:], t2[:], gn[:, c0:c0 + BC])
        n = small.tile([E, BC], F32, tag="n", name="n")
        nc.scalar.activation(n[:], t3[:], ACTF.Tanh)
        d = small.tile([E, BC], F32, tag="d", name="d")
        nc.vector.tensor_sub(d[:], h[0:E, :], n[:])
        zd = small.tile([E, BC], F32, tag="zd", name="zd")
        nc.vector.tensor_mul(zd[:], rz[:, BC:2 * BC], d[:])
        nc.vector.tensor_add(h[0:E, :], n[:], zd[:])   # h = n + z*(h-n)

    def bil_A():
        # A[e, s*16+b] = sum_h Wbil[s,h,e] * hidden[b,h], one matmul per s
        Ap = psR.tile([E, 512], F32, tag="rp", name="Ap")
        for s in range(NE):
            nc.tensor.matmul(Ap[:, s * BC:(s + 1) * BC],
                             Wbil_sb[:, s * E:(s + 1) * E], h[0:E, :],
                             start=True, stop=True, skip_group_check=True)
        nc.vector.tensor_copy(A_sb[:], Ap[:, 0:NE * BC])

    def scores(blk):
        # score[c] = sum_e A[e, c mod 256] * ET[e, col]; partition-reduce
        # via ones-matmul in f32r. Output columns: Ep 0:256, then
        # nb-major 256 + nb*256 + s*16 + b.
        w = 256 if blk == 0 else 512
        co = 0 if blk == 0 else 256 + (blk - 1) * 512
        ce = blk * 512 + (256 if blk == 0 else 0)
        tmp = small.tile([E, 512], F32R, tag="tmp", name="tmp")
        for u in range(w // 256):
            nc.vector.tensor_mul(tmp[:, u * 256:(u + 1) * 256],
                                 ET[0:E, ce + u * 256:ce + (u + 1) * 256],
                                 A_sb[:])
        rp = psR.tile([E, 512], F32, tag="rp", name="rp")
        nc.tensor.matmul(rp[0:1, 0:w], ones_sb[:], tmp[:, 0:w],
                         start=True, stop=True, skip_group_check=True)
        nc.scalar.copy(out_sb[:, co:co + w], rp[0:1, 0:w])

    # gap schedule: one GRU step per k-slab gap, then A + score backlog
    gap_work = {}
    gaps = [(b, s) for b in range(1, NBLK) for s in range(5)]
    gap_work[gaps[0]] = [gru_init]
    for i in range(NE):
        gap_work[gaps[1 + i]] = [lambda s=i: gru_step(s)]
    gap_work[gaps[17]] = [bil_A]
    gap_work[gaps[18]] = [lambda: scores(0), lambda: scores(1)]
    gap_work[gaps[19]] = [lambda: scores(2), lambda: scores(3)]

    # ---- embed: 6 blocks of 512 rows ----
    for blk in range(NBLK):
        slabs = ([SLAB_SMALL] + SLABS_BIG) if blk == 0 \
            else (SLABS_BIG + [SLAB_SMALL])
        et = psE.tile([E, 512], F32, tag="et", name="et")
        nmm = 0
        for si, (k0, kw) in enumerate(slabs):
            xs = [xnat.tile([128, 2048], BF16, tag=f"xn{st}", name=f"xn{st}")
                  for st in range(4)]
            for st in range(4):
                # gpsimd SWDGE casts f32 -> bf16 in the DMA
                nc.gpsimd.dma_start(xs[st][:, 0:kw],
                                    _block_src(Xc, Xp, Xb, blk, st, k0, kw))
            nj = (kw + 127) // 128
            assert nj % 2 == 0
            jbase = k0 // 128
            for jp in range(nj // 2):
                pt = psT.tile([128, 1024], BF16)
                kjs = []
                for u in range(2):
                    j = jp * 2 + u
                    kj = min(128, kw - j * 128)
                    kjs.append(kj)
                    for st in range(4):
                        nc.tensor.transpose(
                            pt[0:kj, u * 512 + st * 128:u * 512 + (st + 1) * 128],
                            xs[st][:, j * 128:j * 128 + kj],
                            id_sb[:])
                xt = xtp.tile([128, 1024], BF16)
                if kjs[1] == 128:
                    nc.vector.tensor_copy(xt[:], pt[:])
                else:  # last pair: u=1 chunk only has kjs[1] valid rows
                    nc.vector.tensor_copy(xt[:, 0:512], pt[:, 0:512])
                    nc.vector.tensor_copy(xt[0:kjs[1], 512:1024],
                                          pt[0:kjs[1], 512:1024])
                for u in range(2):
                    jg = jbase + jp * 2 + u
                    nc.tensor.matmul(
                        et[:, :],
                        W_sb[0:kjs[u], jg * E:(jg + 1) * E],
                        xt[0:kjs[u], u * 512:u * 512 + 512],
                        start=(nmm == 0), stop=(nmm == NCHUNK - 1),
                        skip_group_check=True)
                    nmm += 1
            # GRU / bilinear work scheduled into this slab gap
            for fn in gap_work.get((blk, si), ()):
                fn()
        # bias + evacuate to ET (ACT queue is short -> fires promptly)
        nc.scalar.add(ET[0:E, blk * 512:(blk + 1) * 512], et[:, :],
                      bemb_sb[:, 0:1])
        if blk >= 4:
            scores(blk)

    nc.sync.dma_start(out_d[:], out_sb[:])


def build():
    import contextlib
    nc = bacc.Bacc("TRN2", target_bir_lowering=False, debug=False,
                   enable_asserts=False, num_devices=N_CORES)
    with tile.TileContext(nc) as tc:
        with contextlib.ExitStack() as ctx:
            _emit(nc, tc, ctx)
    nc.compile()
    return nc


_NC = None


def make_in_maps(Xc, Xp, Xb, W_embed, b_embed, W_ih, W_hh, b_ih, b_hh, W_bil):
    B = Xc.shape[0]
    Xc_r = np.ascontiguousarray(Xc, np.float32).reshape(B, NE, CT)
    Xp_r = np.ascontiguousarray(Xp, np.float32).reshape(B, NE, CT)
    Xb_r = np.ascontiguousarray(Xb, np.float32).reshape(B, NE, NB, CT)

    import ml_dtypes
    W_embed = np.ascontiguousarray(W_embed, np.float32)
    W_ch = np.zeros((128, NCHUNK * E), np.float32)
    for j in range(NCHUNK):
        kj = min(128, CT - j * 128)
        W_ch[:kj, j * E:(j + 1) * E] = W_embed[j * 128:j * 128 + kj]
    W_ch = W_ch.astype(ml_dtypes.bfloat16)
    bemb = np.ascontiguousarray(b_embed, np.float32).reshape(E, 1)
    b_ih = np.asarray(b_ih, np.float32)
    b_hh = np.asarray(b_hh, np.float32)
    # augmented weights: row 100 is the bias row hit by the ones row of
    # the rhs (ET / h).  gi gets b_ih (+ b_hh for r,z); gh_n gets b_hn.
    wih_bias = np.concatenate([b_ih[0:2 * E] + b_hh[0:2 * E], b_ih[2 * E:]])
    WihT = np.concatenate([np.asarray(W_ih, np.float32).T,
                           wih_bias[None, :]], axis=0)
    whh_bias = np.concatenate([np.zeros(2 * E, np.float32), b_hh[2 * E:]])
    WhhT = np.concatenate([np.asarray(W_hh, np.float32).T,
                           whh_bias[None, :]], axis=0)
    Wbil_r = np.ascontiguousarray(
        np.transpose(W_bil, (1, 0, 2)).reshape(E, NE * E), np.float32)
    ident = np.eye(128).astype(ml_dtypes.bfloat16)
    ones = np.ones((E, 1), np.float32)

    shared = dict(Wemb=W_ch, bemb=bemb, WihT=WihT, WhhT=WhhT,
                  Wbil=Wbil_r, ident=ident, ones=ones,
                  onesrow=np.ones((1, ROWS), np.float32))
    in_maps = []
    for c in range(N_CORES):
        sl = slice(c * BC, (c + 1) * BC)
        in_maps.append(dict(Xc=Xc_r[sl], Xp=Xp_r[sl], Xb=Xb_r[sl], **shared))
    return in_maps


def gather(results):
    outs = []
    for c in range(N_CORES):
        flat = results[c]["out"].reshape(NOUT)
        o = np.empty((BC, NE, NB + 1), np.float32)
        o[:, :, 0] = flat[0:256].reshape(NE, BC).T
        o[:, :, 1:] = flat[256:].reshape(NB, NE, BC).transpose(2, 1, 0)
        outs.append(o)
    return np.concatenate(outs, axis=0).astype(np.float32)  # [128, 16, 11]


def kernel(Xc, Xp, Xb, W_embed, b_embed, W_ih, W_hh, b_ih, b_hh, W_bil):
    global _NC
    if _NC is None:
        _NC = build()
    in_maps = make_in_maps(Xc, Xp, Xb, W_embed, b_embed, W_ih, W_hh,
                           b_ih, b_hh, W_bil)
    res = run_bass_kernel_spmd(_NC, in_maps, core_ids=list(range(N_CORES)))
    return gather(res.results)
